# revision 5
# baseline (speedup 1.0000x reference)
"""Grouped SwiGLU experts (MoE post-dispatch compute) on 8 Trainium2 cores.

Expert-parallel: host gathers tokens per expert (the "all-to-all dispatch"),
packs 128-token blocks into a uniform per-core segment schedule (specialized
to the actual counts at compile time), each segment runs one expert's SwiGLU
  hT = silu(w1.T x.T) * (w3.T x.T);  out = (hT.T) @ w2
on one NeuronCore in bf16 with fp32 PSUM accumulation, and the host
scatters rows back to their original token positions.
"""

import numpy as np
import ml_dtypes

# ---- problem constants (from the reference module) ----
T, D, H, E, R, ALIGN = 8192, 4096, 1024, 8, 2, 16
P = 128          # partition width
DT = D // P      # 32 d-tiles
HT = H // P      # 8 h-chunks
NCORES = 8
MAX_PART = 4     # max 128-blocks per segment (keeps psum tile <= 1 bank)

BF16 = ml_dtypes.bfloat16


def _permute_indices(counts):
    """numpy port of reference._permute_indices."""
    counts = counts.astype(np.int64)
    max_len = T + E * ALIGN
    start_index = np.cumsum(counts) - counts
    total = counts.reshape(R, E).sum(0)
    m_sizes = ((np.maximum(total, ALIGN) + ALIGN - 1) // ALIGN * ALIGN).astype(np.int64)
    m_offsets = np.cumsum(m_sizes)
    write_offsets = m_offsets - m_sizes
    c_er = counts.reshape(R, E).T
    seg_ws = (write_offsets[:, None] + np.cumsum(c_er, 1) - c_er).reshape(-1)
    seg_len = c_er.reshape(-1)
    seg_src = start_index.reshape(R, E).T.reshape(-1)
    pos = np.arange(max_len, dtype=np.int64)
    idx = np.clip(np.searchsorted(seg_ws, pos, side="right") - 1, 0, E * R - 1)
    within = pos - seg_ws[idx]
    valid = (within >= 0) & (within < seg_len[idx])
    perm = np.where(valid, seg_src[idx] + within, T)
    return perm.astype(np.int64), m_sizes, (m_offsets - m_sizes)


def _partitions(c, max_part, max_len):
    """Partitions of c into <= max_len parts each <= max_part, desc order."""
    out = []

    def rec(rem, mx, cur):
        if rem == 0:
            out.append(tuple(cur))
            return
        if len(cur) == max_len:
            return
        for p in range(min(rem, mx), 0, -1):
            cur.append(p)
            rec(rem - p, p, cur)
            cur.pop()

    rec(c, max_part, [])
    return out


def _ffd(nblk, parts):
    """Pack per-expert block counts into 8 cores x len(parts) bins
    (bin (c,j) capacity parts[j], single expert per bin). Returns
    assignment dict (core, j) -> (expert, block_start, nb) or None."""
    bins = []  # (cap, core, j)
    for c in range(NCORES):
        for j, cap in enumerate(parts):
            bins.append([cap, c, j])
    bins.sort(key=lambda b: -b[0])
    used = [False] * len(bins)
    asg = {}
    order = sorted(range(E), key=lambda e: -nblk[e])
    for e in order:
        rem = int(nblk[e])
        b0 = 0
        while rem > 0:
            # largest unused bin with cap <= rem (fill fully); else the
            # smallest unused bin (minimize slack)
            best_le, best_gt = None, None
            for i, (cap, c, j) in enumerate(bins):
                if used[i]:
                    continue
                if cap <= rem:
                    best_le = i  # bins desc: first such is largest
                    break
                best_gt = i  # keeps updating: last seen = smallest so far
            i = best_le if best_le is not None else best_gt
            if i is None:
                return None
            cap, c, j = bins[i]
            used[i] = True
            nb = min(cap, rem)
            asg[(c, j)] = (e, b0, nb)
            b0 += nb
            rem -= nb
    return asg


def _plan(nblk):
    n = int(sum(nblk))
    c0 = max(1, -(-n // NCORES))
    for c in range(c0, 70):
        opts = _partitions(c, MAX_PART, 3)
        # prefer fewer segments, then most balanced (large min part)
        for parts in sorted(opts, key=lambda p: (len(p), -min(p))):
            asg = _ffd(nblk, parts)
            if asg is not None:
                return list(parts), asg
    raise RuntimeError("packing failed")


def _build_program(parts):
    import concourse.mybir as mybir
    import concourse.tile as tile
    from concourse import bacc

    bf = mybir.dt.bfloat16
    f32 = mybir.dt.float32
    SILU = mybir.ActivationFunctionType.Silu

    nc = bacc.Bacc("TRN2", target_bir_lowering=False, debug=False,
                   num_devices=NCORES)

    xt_d, w1_d, w3_d, w2_d, out_d = [], [], [], [], []
    for j, B in enumerate(parts):
        M = B * P
        xt_d.append(nc.dram_tensor(f"xt{j}", [DT, P, M], bf, kind="ExternalInput"))
        w1_d.append(nc.dram_tensor(f"w1p{j}", [HT, P, D], bf, kind="ExternalInput"))
        w3_d.append(nc.dram_tensor(f"w3p{j}", [HT, P, D], bf, kind="ExternalInput"))
        w2_d.append(nc.dram_tensor(f"w2p{j}", [H, D], bf, kind="ExternalInput"))
        out_d.append(nc.dram_tensor(f"out{j}", [M, D], f32, kind="ExternalOutput"))

    with tile.TileContext(nc) as tc:
        with (
            tc.tile_pool(name="xt", bufs=2 * DT) as xt_pool,
            tc.tile_pool(name="wp", bufs=4) as wp_pool,
            tc.tile_pool(name="w2", bufs=HT) as w2_pool,
            tc.tile_pool(name="ht", bufs=2 * HT) as ht_pool,
            tc.tile_pool(name="stmp", bufs=2) as stmp_pool,
            tc.tile_pool(name="ost", bufs=4) as ost_pool,
            tc.tile_pool(name="ps1", bufs=2, space="PSUM") as ps1_pool,
            tc.tile_pool(name="ps3", bufs=2, space="PSUM") as ps3_pool,
            tc.tile_pool(name="pso", bufs=2, space="PSUM") as pso_pool,
        ):
            for j, B in enumerate(parts):
                M = B * P
                xts = []
                for d in range(DT):
                    t = xt_pool.tile([P, M], bf, tag="xt")
                    nc.sync.dma_start(out=t[:], in_=xt_d[j][d])
                    xts.append(t)
                w2s = []
                for h in range(HT):
                    t = w2_pool.tile([P, D], bf, tag="w2")
                    nc.sync.dma_start(out=t[:], in_=w2_d[j][h * P:(h + 1) * P, :])
                    w2s.append(t)
                hts = [ht_pool.tile([P, M], bf, tag="ht", name=f"ht{j}_{h}")
                       for h in range(HT)]
                for h in range(HT):
                    w1p = wp_pool.tile([P, D], bf, tag="wp")
                    nc.sync.dma_start(out=w1p[:], in_=w1_d[j][h])
                    w3p = wp_pool.tile([P, D], bf, tag="wp")
                    nc.sync.dma_start(out=w3p[:], in_=w3_d[j][h])
                    ps1 = ps1_pool.tile([P, M], f32, tag="ps1")
                    ps3 = ps3_pool.tile([P, M], f32, tag="ps3")
                    for d in range(DT):
                        nc.tensor.matmul(ps1[:], w1p[:, d * P:(d + 1) * P],
                                         xts[d][:], start=(d == 0),
                                         stop=(d == DT - 1))
                        nc.tensor.matmul(ps3[:], w3p[:, d * P:(d + 1) * P],
                                         xts[d][:], start=(d == 0),
                                         stop=(d == DT - 1))
                    tmp = stmp_pool.tile([P, M], f32, tag="stmp")
                    nc.scalar.activation(tmp[:], ps1[:], SILU)
                    nc.vector.tensor_mul(hts[h][:], tmp[:], ps3[:])
                for b in range(B):
                    for dc in range(D // 512):
                        po = pso_pool.tile([P, 512], f32, tag="pso")
                        for h in range(HT):
                            nc.tensor.matmul(
                                po[:], hts[h][:, b * P:(b + 1) * P],
                                w2s[h][:, dc * 512:(dc + 1) * 512],
                                start=(h == 0), stop=(h == HT - 1))
                        ob = ost_pool.tile([P, 512], f32, tag="ost")
                        nc.vector.tensor_copy(ob[:], po[:])
                        nc.sync.dma_start(
                            out=out_d[j][b * P:(b + 1) * P, dc * 512:(dc + 1) * 512],
                            in_=ob[:])

    nc.compile()
    return nc


_CACHE = {}


def _get_program(parts):
    key = tuple(parts)
    if key not in _CACHE:
        _CACHE[key] = _build_program(parts)
    return _CACHE[key]


_LAST_RESULT = None


def kernel(x, w1, w2, w3, num_tokens_per_expert):
    import os
    from concourse.bass_utils import run_bass_kernel_spmd

    x = np.asarray(x, dtype=np.float32)
    counts = np.asarray(num_tokens_per_expert).astype(np.int64)
    perm, m_sizes, m_off = _permute_indices(counts)
    nblk = (m_sizes + P - 1) // P

    parts, asg = _plan(nblk)
    nc = _get_program(parts)

    # expert-grouped token stream (the dispatch): rows of x per expert
    x_pad = np.concatenate([x, np.zeros((1, D), np.float32)], axis=0)
    ltot = int(m_sizes.sum())
    xp = x_pad[perm[:ltot]]  # [ltot, D] expert-grouped, 16-aligned per expert

    # per-expert blocks, zero-padded to nblk[e]*128 rows
    xe = []
    for e in range(E):
        rows = xp[m_off[e]:m_off[e] + m_sizes[e]]
        padr = int(nblk[e] * P - m_sizes[e])
        if padr:
            rows = np.concatenate([rows, np.zeros((padr, D), np.float32)], 0)
        xe.append(rows)

    w1b = [np.ascontiguousarray(
        np.asarray(w1[e], np.float32).reshape(DT, P, HT, P)
        .transpose(2, 1, 0, 3).reshape(HT, P, D)).astype(BF16) for e in range(E)]
    w3b = [np.ascontiguousarray(
        np.asarray(w3[e], np.float32).reshape(DT, P, HT, P)
        .transpose(2, 1, 0, 3).reshape(HT, P, D)).astype(BF16) for e in range(E)]
    w2b = [np.asarray(w2[e], np.float32).astype(BF16) for e in range(E)]

    in_maps = []
    for c in range(NCORES):
        m = {}
        for j, B in enumerate(parts):
            M = B * P
            ent = asg.get((c, j))
            e = ent[0] if ent is not None else 0
            blk = np.zeros((M, D), np.float32)
            if ent is not None:
                _, b0, nb = ent
                blk[:nb * P] = xe[e][b0 * P:(b0 + nb) * P]
            # [DT, P, M]: xt[d, p, m] = blk[m, d*P+p]
            m[f"xt{j}"] = np.ascontiguousarray(
                blk.reshape(M, DT, P).transpose(1, 2, 0)).astype(BF16)
            m[f"w1p{j}"] = w1b[e]
            m[f"w3p{j}"] = w3b[e]
            m[f"w2p{j}"] = w2b[e]
        in_maps.append(m)

    kw = {}
    if os.environ.get("KERNEL_TRACE"):
        kw = dict(trace=True, tmpdir=os.environ.get("KERNEL_TRACE_DIR") or None)
    res = run_bass_kernel_spmd(nc, in_maps, core_ids=list(range(NCORES)), **kw)
    global _LAST_RESULT
    _LAST_RESULT = res

    # reassemble expert-grouped output stream, then scatter to token order
    outp = np.zeros((ltot, D), np.float32)
    for (c, j), (e, b0, nb) in asg.items():
        rows = res.results[c][f"out{j}"][:nb * P]
        s = m_off[e] + b0 * P
        stop = min(int(m_off[e] + m_sizes[e]), int(s + nb * P))
        if stop > s:
            outp[s:stop] = rows[:stop - s]

    out = np.zeros((T + 1, D), np.float32)
    out[perm[:ltot]] = outp
    return out[:T]


# revision 8
# speedup vs baseline: 1.1598x; 1.1598x over previous
"""Grouped SwiGLU experts (MoE post-dispatch compute) on 8 Trainium2 cores.

Expert-parallel: host gathers tokens per expert (the "all-to-all dispatch"),
packs 128-token blocks into a uniform per-core segment schedule (specialized
to the actual counts at compile time), each segment runs one expert's SwiGLU
  hT = silu(w1.T x.T) * (w3.T x.T);  out = (hT.T) @ w2
on one NeuronCore in bf16 with fp32 PSUM accumulation, and the host
scatters rows back to their original token positions.
"""

import numpy as np
import ml_dtypes

# ---- problem constants (from the reference module) ----
T, D, H, E, R, ALIGN = 8192, 4096, 1024, 8, 2, 16
P = 128          # partition width
DT = D // P      # 32 d-tiles
HT = H // P      # 8 h-chunks
NCORES = 8
MAX_PART = 4     # max 128-blocks per segment (keeps psum tile <= 1 bank)

BF16 = ml_dtypes.bfloat16


def _permute_indices(counts):
    """numpy port of reference._permute_indices."""
    counts = counts.astype(np.int64)
    max_len = T + E * ALIGN
    start_index = np.cumsum(counts) - counts
    total = counts.reshape(R, E).sum(0)
    m_sizes = ((np.maximum(total, ALIGN) + ALIGN - 1) // ALIGN * ALIGN).astype(np.int64)
    m_offsets = np.cumsum(m_sizes)
    write_offsets = m_offsets - m_sizes
    c_er = counts.reshape(R, E).T
    seg_ws = (write_offsets[:, None] + np.cumsum(c_er, 1) - c_er).reshape(-1)
    seg_len = c_er.reshape(-1)
    seg_src = start_index.reshape(R, E).T.reshape(-1)
    pos = np.arange(max_len, dtype=np.int64)
    idx = np.clip(np.searchsorted(seg_ws, pos, side="right") - 1, 0, E * R - 1)
    within = pos - seg_ws[idx]
    valid = (within >= 0) & (within < seg_len[idx])
    perm = np.where(valid, seg_src[idx] + within, T)
    return perm.astype(np.int64), m_sizes, (m_offsets - m_sizes)


def _partitions(c, max_part, max_len):
    """Partitions of c into <= max_len parts each <= max_part, desc order."""
    out = []

    def rec(rem, mx, cur):
        if rem == 0:
            out.append(tuple(cur))
            return
        if len(cur) == max_len:
            return
        for p in range(min(rem, mx), 0, -1):
            cur.append(p)
            rec(rem - p, p, cur)
            cur.pop()

    rec(c, max_part, [])
    return out


def _ffd(nblk, parts):
    """Pack per-expert block counts into 8 cores x len(parts) bins
    (bin (c,j) capacity parts[j], single expert per bin). Returns
    assignment dict (core, j) -> (expert, block_start, nb) or None."""
    bins = []  # (cap, core, j)
    for c in range(NCORES):
        for j, cap in enumerate(parts):
            bins.append([cap, c, j])
    bins.sort(key=lambda b: -b[0])
    used = [False] * len(bins)
    asg = {}
    order = sorted(range(E), key=lambda e: -nblk[e])
    for e in order:
        rem = int(nblk[e])
        b0 = 0
        while rem > 0:
            # largest unused bin with cap <= rem (fill fully); else the
            # smallest unused bin (minimize slack)
            best_le, best_gt = None, None
            for i, (cap, c, j) in enumerate(bins):
                if used[i]:
                    continue
                if cap <= rem:
                    best_le = i  # bins desc: first such is largest
                    break
                best_gt = i  # keeps updating: last seen = smallest so far
            i = best_le if best_le is not None else best_gt
            if i is None:
                return None
            cap, c, j = bins[i]
            used[i] = True
            nb = min(cap, rem)
            asg[(c, j)] = (e, b0, nb)
            b0 += nb
            rem -= nb
    return asg


def _plan(nblk):
    n = int(sum(nblk))
    c0 = max(1, -(-n // NCORES))
    for c in range(c0, 70):
        opts = _partitions(c, MAX_PART, 3)
        # prefer fewer segments, then most balanced (large min part)
        for parts in sorted(opts, key=lambda p: (len(p), -min(p))):
            asg = _ffd(nblk, parts)
            if asg is not None:
                return list(parts), asg
    raise RuntimeError("packing failed")


def _build_program(parts):
    import concourse.mybir as mybir
    import concourse.tile as tile
    from concourse import bacc

    bf = mybir.dt.bfloat16
    f32 = mybir.dt.float32
    SILU = mybir.ActivationFunctionType.Silu

    nc = bacc.Bacc("TRN2", target_bir_lowering=False, debug=False,
                   num_devices=NCORES)

    xt_d, w1_d, w3_d, w2_d, out_d = [], [], [], [], []
    for j, B in enumerate(parts):
        M = B * P
        xt_d.append(nc.dram_tensor(f"xt{j}", [DT, P, M], bf, kind="ExternalInput"))
        w1_d.append(nc.dram_tensor(f"w1p{j}", [HT, P, D], bf, kind="ExternalInput"))
        w3_d.append(nc.dram_tensor(f"w3p{j}", [HT, P, D], bf, kind="ExternalInput"))
        w2_d.append(nc.dram_tensor(f"w2p{j}", [H, D], bf, kind="ExternalInput"))
        out_d.append(nc.dram_tensor(f"out{j}", [M, D], f32, kind="ExternalOutput"))

    with tile.TileContext(nc) as tc:
        with (
            tc.tile_pool(name="xt", bufs=2 * DT) as xt_pool,
            tc.tile_pool(name="wp", bufs=6) as wp_pool,
            tc.tile_pool(name="w2", bufs=HT) as w2_pool,
            tc.tile_pool(name="ht", bufs=2 * HT) as ht_pool,
            tc.tile_pool(name="stmp", bufs=2) as stmp_pool,
            tc.tile_pool(name="ost", bufs=4) as ost_pool,
            tc.tile_pool(name="ps1", bufs=2, space="PSUM") as ps1_pool,
            tc.tile_pool(name="ps3", bufs=2, space="PSUM") as ps3_pool,
            tc.tile_pool(name="pso", bufs=2, space="PSUM") as pso_pool,
        ):
            for j, B in enumerate(parts):
                M = B * P

                def load_wp(h, j=j):
                    t1 = wp_pool.tile([P, D], bf, tag="wp", name=f"w1p{j}_{h}")
                    nc.sync.dma_start(out=t1[:], in_=w1_d[j][h])
                    t3 = wp_pool.tile([P, D], bf, tag="wp", name=f"w3p{j}_{h}")
                    nc.sync.dma_start(out=t3[:], in_=w3_d[j][h])
                    return t1, t3

                # critical path first: h=0 weight panels, then token tiles;
                # w2 (phase 2 only) is deferred until after phase 1 emission
                wp_cur = load_wp(0)
                xts = []
                for d in range(DT):
                    t = xt_pool.tile([P, M], bf, tag="xt", name=f"xt{j}_{d}")
                    nc.sync.dma_start(out=t[:], in_=xt_d[j][d])
                    xts.append(t)
                hts = [ht_pool.tile([P, M], bf, tag="ht", name=f"ht{j}_{h}")
                       for h in range(HT)]
                for h in range(HT):
                    w1p, w3p = wp_cur
                    if h + 1 < HT:
                        wp_cur = load_wp(h + 1)
                    ps1 = ps1_pool.tile([P, M], f32, tag="ps1")
                    ps3 = ps3_pool.tile([P, M], f32, tag="ps3")
                    for d in range(DT):
                        nc.tensor.matmul(ps1[:], w1p[:, d * P:(d + 1) * P],
                                         xts[d][:], start=(d == 0),
                                         stop=(d == DT - 1))
                        nc.tensor.matmul(ps3[:], w3p[:, d * P:(d + 1) * P],
                                         xts[d][:], start=(d == 0),
                                         stop=(d == DT - 1))
                    tmp = stmp_pool.tile([P, M], f32, tag="stmp")
                    nc.scalar.activation(tmp[:], ps1[:], SILU)
                    nc.vector.tensor_mul(hts[h][:], tmp[:], ps3[:])
                w2s = []
                for h in range(HT):
                    t = w2_pool.tile([P, D], bf, tag="w2", name=f"w2{j}_{h}")
                    nc.sync.dma_start(out=t[:], in_=w2_d[j][h * P:(h + 1) * P, :])
                    w2s.append(t)
                for b in range(B):
                    for dc in range(D // 512):
                        po = pso_pool.tile([P, 512], f32, tag="pso")
                        for h in range(HT):
                            nc.tensor.matmul(
                                po[:], hts[h][:, b * P:(b + 1) * P],
                                w2s[h][:, dc * 512:(dc + 1) * 512],
                                start=(h == 0), stop=(h == HT - 1))
                        ob = ost_pool.tile([P, 512], f32, tag="ost")
                        nc.vector.tensor_copy(ob[:], po[:])
                        nc.sync.dma_start(
                            out=out_d[j][b * P:(b + 1) * P, dc * 512:(dc + 1) * 512],
                            in_=ob[:])

    nc.compile()
    return nc


_CACHE = {}


def _get_program(parts):
    key = tuple(parts)
    if key not in _CACHE:
        _CACHE[key] = _build_program(parts)
    return _CACHE[key]


_LAST_RESULT = None


def kernel(x, w1, w2, w3, num_tokens_per_expert):
    import os
    from concourse.bass_utils import run_bass_kernel_spmd

    x = np.asarray(x, dtype=np.float32)
    counts = np.asarray(num_tokens_per_expert).astype(np.int64)
    perm, m_sizes, m_off = _permute_indices(counts)
    nblk = (m_sizes + P - 1) // P

    parts, asg = _plan(nblk)
    nc = _get_program(parts)

    # expert-grouped token stream (the dispatch): rows of x per expert
    x_pad = np.concatenate([x, np.zeros((1, D), np.float32)], axis=0)
    ltot = int(m_sizes.sum())
    xp = x_pad[perm[:ltot]]  # [ltot, D] expert-grouped, 16-aligned per expert

    # per-expert blocks, zero-padded to nblk[e]*128 rows
    xe = []
    for e in range(E):
        rows = xp[m_off[e]:m_off[e] + m_sizes[e]]
        padr = int(nblk[e] * P - m_sizes[e])
        if padr:
            rows = np.concatenate([rows, np.zeros((padr, D), np.float32)], 0)
        xe.append(rows)

    w1b = [np.ascontiguousarray(
        np.asarray(w1[e], np.float32).reshape(DT, P, HT, P)
        .transpose(2, 1, 0, 3).reshape(HT, P, D)).astype(BF16) for e in range(E)]
    w3b = [np.ascontiguousarray(
        np.asarray(w3[e], np.float32).reshape(DT, P, HT, P)
        .transpose(2, 1, 0, 3).reshape(HT, P, D)).astype(BF16) for e in range(E)]
    w2b = [np.asarray(w2[e], np.float32).astype(BF16) for e in range(E)]

    in_maps = []
    for c in range(NCORES):
        m = {}
        for j, B in enumerate(parts):
            M = B * P
            ent = asg.get((c, j))
            e = ent[0] if ent is not None else 0
            blk = np.zeros((M, D), np.float32)
            if ent is not None:
                _, b0, nb = ent
                blk[:nb * P] = xe[e][b0 * P:(b0 + nb) * P]
            # [DT, P, M]: xt[d, p, m] = blk[m, d*P+p]
            m[f"xt{j}"] = np.ascontiguousarray(
                blk.reshape(M, DT, P).transpose(1, 2, 0)).astype(BF16)
            m[f"w1p{j}"] = w1b[e]
            m[f"w3p{j}"] = w3b[e]
            m[f"w2p{j}"] = w2b[e]
        in_maps.append(m)

    kw = {}
    if os.environ.get("KERNEL_TRACE"):
        kw = dict(trace=True, tmpdir=os.environ.get("KERNEL_TRACE_DIR") or None)
    res = run_bass_kernel_spmd(nc, in_maps, core_ids=list(range(NCORES)), **kw)
    global _LAST_RESULT
    _LAST_RESULT = res

    # reassemble expert-grouped output stream, then scatter to token order
    outp = np.zeros((ltot, D), np.float32)
    for (c, j), (e, b0, nb) in asg.items():
        rows = res.results[c][f"out{j}"][:nb * P]
        s = m_off[e] + b0 * P
        stop = min(int(m_off[e] + m_sizes[e]), int(s + nb * P))
        if stop > s:
            outp[s:stop] = rows[:stop - s]

    out = np.zeros((T + 1, D), np.float32)
    out[perm[:ltot]] = outp
    return out[:T]


# revision 11
# speedup vs baseline: 1.1781x; 1.0158x over previous
"""Grouped SwiGLU experts (MoE post-dispatch compute) on 8 Trainium2 cores.

Expert-parallel: host gathers tokens per expert (the "all-to-all dispatch"),
packs 128-token blocks into a uniform per-core segment schedule (specialized
to the actual counts at compile time), each segment runs one expert's SwiGLU
  hT = silu(w1.T x.T) * (w3.T x.T);  out = (hT.T) @ w2
on one NeuronCore in bf16 with fp32 PSUM accumulation, and the host
scatters rows back to their original token positions.
"""

import numpy as np
import ml_dtypes

# ---- problem constants (from the reference module) ----
T, D, H, E, R, ALIGN = 8192, 4096, 1024, 8, 2, 16
P = 128          # partition width
DT = D // P      # 32 d-tiles
HT = H // P      # 8 h-chunks
NCORES = 8
MAX_PART = 4     # max 128-blocks per segment (keeps psum tile <= 1 bank)

BF16 = ml_dtypes.bfloat16


def _permute_indices(counts):
    """numpy port of reference._permute_indices."""
    counts = counts.astype(np.int64)
    max_len = T + E * ALIGN
    start_index = np.cumsum(counts) - counts
    total = counts.reshape(R, E).sum(0)
    m_sizes = ((np.maximum(total, ALIGN) + ALIGN - 1) // ALIGN * ALIGN).astype(np.int64)
    m_offsets = np.cumsum(m_sizes)
    write_offsets = m_offsets - m_sizes
    c_er = counts.reshape(R, E).T
    seg_ws = (write_offsets[:, None] + np.cumsum(c_er, 1) - c_er).reshape(-1)
    seg_len = c_er.reshape(-1)
    seg_src = start_index.reshape(R, E).T.reshape(-1)
    pos = np.arange(max_len, dtype=np.int64)
    idx = np.clip(np.searchsorted(seg_ws, pos, side="right") - 1, 0, E * R - 1)
    within = pos - seg_ws[idx]
    valid = (within >= 0) & (within < seg_len[idx])
    perm = np.where(valid, seg_src[idx] + within, T)
    return perm.astype(np.int64), m_sizes, (m_offsets - m_sizes)


def _partitions(c, max_part, max_len):
    """Partitions of c into <= max_len parts each <= max_part, desc order."""
    out = []

    def rec(rem, mx, cur):
        if rem == 0:
            out.append(tuple(cur))
            return
        if len(cur) == max_len:
            return
        for p in range(min(rem, mx), 0, -1):
            cur.append(p)
            rec(rem - p, p, cur)
            cur.pop()

    rec(c, max_part, [])
    return out


def _ffd(nblk, parts):
    """Pack per-expert block counts into 8 cores x len(parts) bins
    (bin (c,j) capacity parts[j], single expert per bin). Returns
    assignment dict (core, j) -> (expert, block_start, nb) or None."""
    bins = []  # (cap, core, j)
    for c in range(NCORES):
        for j, cap in enumerate(parts):
            bins.append([cap, c, j])
    bins.sort(key=lambda b: -b[0])
    used = [False] * len(bins)
    asg = {}
    order = sorted(range(E), key=lambda e: -nblk[e])
    for e in order:
        rem = int(nblk[e])
        b0 = 0
        while rem > 0:
            # largest unused bin with cap <= rem (fill fully); else the
            # smallest unused bin (minimize slack)
            best_le, best_gt = None, None
            for i, (cap, c, j) in enumerate(bins):
                if used[i]:
                    continue
                if cap <= rem:
                    best_le = i  # bins desc: first such is largest
                    break
                best_gt = i  # keeps updating: last seen = smallest so far
            i = best_le if best_le is not None else best_gt
            if i is None:
                return None
            cap, c, j = bins[i]
            used[i] = True
            nb = min(cap, rem)
            asg[(c, j)] = (e, b0, nb)
            b0 += nb
            rem -= nb
    return asg


def _plan(nblk):
    n = int(sum(nblk))
    c0 = max(1, -(-n // NCORES))
    for c in range(c0, 70):
        opts = _partitions(c, MAX_PART, 3)
        # prefer fewer segments, then most balanced (large min part)
        for parts in sorted(opts, key=lambda p: (len(p), -min(p))):
            asg = _ffd(nblk, parts)
            if asg is not None:
                return list(parts), asg
    raise RuntimeError("packing failed")


def _build_program(parts):
    import concourse.mybir as mybir
    import concourse.tile as tile
    from concourse import bacc

    bf = mybir.dt.bfloat16
    f32 = mybir.dt.float32
    SILU = mybir.ActivationFunctionType.Silu

    nc = bacc.Bacc("TRN2", target_bir_lowering=False, debug=False,
                   num_devices=NCORES)

    xt_d, w1_d, w3_d, w2_d, out_d = [], [], [], [], []
    for j, B in enumerate(parts):
        M = B * P
        xt_d.append(nc.dram_tensor(f"xt{j}", [DT, P, M], bf, kind="ExternalInput"))
        w1_d.append(nc.dram_tensor(f"w1p{j}", [HT, P, D], bf, kind="ExternalInput"))
        w3_d.append(nc.dram_tensor(f"w3p{j}", [HT, P, D], bf, kind="ExternalInput"))
        w2_d.append(nc.dram_tensor(f"w2p{j}", [H, D], bf, kind="ExternalInput"))
        out_d.append(nc.dram_tensor(f"out{j}", [M, D], f32, kind="ExternalOutput"))

    with tile.TileContext(nc) as tc:
        with (
            tc.tile_pool(name="xt", bufs=2 * DT) as xt_pool,
            tc.tile_pool(name="wp", bufs=12) as wp_pool,
            tc.tile_pool(name="w2", bufs=HT) as w2_pool,
            tc.tile_pool(name="ht", bufs=2 * HT) as ht_pool,
            tc.tile_pool(name="stmp", bufs=2) as stmp_pool,
            tc.tile_pool(name="ost", bufs=6) as ost_pool,
            tc.tile_pool(name="ps1", bufs=2, space="PSUM") as ps1_pool,
            tc.tile_pool(name="ps3", bufs=2, space="PSUM") as ps3_pool,
            tc.tile_pool(name="pso", bufs=3, space="PSUM") as pso_pool,
        ):
            for j, B in enumerate(parts):
                M = B * P

                def load_wp(h, j=j):
                    # half-panels: first matmuls only wait on 512KB, and
                    # panel loads pipeline at finer grain
                    DH = D // 2
                    tiles = []
                    for half in range(2):
                        t1 = wp_pool.tile([P, DH], bf, tag="wp",
                                          name=f"w1p{j}_{h}_{half}")
                        nc.sync.dma_start(
                            out=t1[:], in_=w1_d[j][h, :, half * DH:(half + 1) * DH])
                        t3 = wp_pool.tile([P, DH], bf, tag="wp",
                                          name=f"w3p{j}_{h}_{half}")
                        nc.sync.dma_start(
                            out=t3[:], in_=w3_d[j][h, :, half * DH:(half + 1) * DH])
                        tiles.append((t1, t3))
                    return tiles

                # critical path first: h=0 weight panels, then token tiles;
                # w2 (phase 2 only) is deferred until after phase 1 emission
                wp_cur = load_wp(0)
                xts = []
                for d in range(DT):
                    t = xt_pool.tile([P, M], bf, tag="xt", name=f"xt{j}_{d}")
                    nc.sync.dma_start(out=t[:], in_=xt_d[j][d])
                    xts.append(t)
                hts = [ht_pool.tile([P, M], bf, tag="ht", name=f"ht{j}_{h}")
                       for h in range(HT)]
                for h in range(HT):
                    wp_halves = wp_cur
                    if h + 1 < HT:
                        wp_cur = load_wp(h + 1)
                    ps1 = ps1_pool.tile([P, M], f32, tag="ps1")
                    ps3 = ps3_pool.tile([P, M], f32, tag="ps3")
                    DH = DT // 2
                    for d in range(DT):
                        w1p, w3p = wp_halves[d // DH]
                        dd = d % DH
                        nc.tensor.matmul(ps1[:], w1p[:, dd * P:(dd + 1) * P],
                                         xts[d][:], start=(d == 0),
                                         stop=(d == DT - 1))
                        nc.tensor.matmul(ps3[:], w3p[:, dd * P:(dd + 1) * P],
                                         xts[d][:], start=(d == 0),
                                         stop=(d == DT - 1))
                    tmp = stmp_pool.tile([P, M], f32, tag="stmp")
                    nc.scalar.activation(tmp[:], ps1[:], SILU)
                    nc.vector.tensor_mul(hts[h][:], tmp[:], ps3[:])
                w2s = []
                for h in range(HT):
                    t = w2_pool.tile([P, D], bf, tag="w2", name=f"w2{j}_{h}")
                    nc.sync.dma_start(out=t[:], in_=w2_d[j][h * P:(h + 1) * P, :])
                    w2s.append(t)
                for b in range(B):
                    for dc in range(D // 512):
                        po = pso_pool.tile([P, 512], f32, tag="pso")
                        for h in range(HT):
                            nc.tensor.matmul(
                                po[:], hts[h][:, b * P:(b + 1) * P],
                                w2s[h][:, dc * 512:(dc + 1) * 512],
                                start=(h == 0), stop=(h == HT - 1))
                        ob = ost_pool.tile([P, 512], f32, tag="ost")
                        nc.vector.tensor_copy(ob[:], po[:])
                        nc.sync.dma_start(
                            out=out_d[j][b * P:(b + 1) * P, dc * 512:(dc + 1) * 512],
                            in_=ob[:])

    nc.compile()
    return nc


_CACHE = {}


def _get_program(parts):
    key = tuple(parts)
    if key not in _CACHE:
        _CACHE[key] = _build_program(parts)
    return _CACHE[key]


_LAST_RESULT = None


def kernel(x, w1, w2, w3, num_tokens_per_expert):
    import os
    from concourse.bass_utils import run_bass_kernel_spmd

    x = np.asarray(x, dtype=np.float32)
    counts = np.asarray(num_tokens_per_expert).astype(np.int64)
    perm, m_sizes, m_off = _permute_indices(counts)
    nblk = (m_sizes + P - 1) // P

    parts, asg = _plan(nblk)
    nc = _get_program(parts)

    # expert-grouped token stream (the dispatch): rows of x per expert
    x_pad = np.concatenate([x, np.zeros((1, D), np.float32)], axis=0)
    ltot = int(m_sizes.sum())
    xp = x_pad[perm[:ltot]]  # [ltot, D] expert-grouped, 16-aligned per expert

    # per-expert blocks, zero-padded to nblk[e]*128 rows
    xe = []
    for e in range(E):
        rows = xp[m_off[e]:m_off[e] + m_sizes[e]]
        padr = int(nblk[e] * P - m_sizes[e])
        if padr:
            rows = np.concatenate([rows, np.zeros((padr, D), np.float32)], 0)
        xe.append(rows)

    w1b = [np.ascontiguousarray(
        np.asarray(w1[e], np.float32).reshape(DT, P, HT, P)
        .transpose(2, 1, 0, 3).reshape(HT, P, D)).astype(BF16) for e in range(E)]
    w3b = [np.ascontiguousarray(
        np.asarray(w3[e], np.float32).reshape(DT, P, HT, P)
        .transpose(2, 1, 0, 3).reshape(HT, P, D)).astype(BF16) for e in range(E)]
    w2b = [np.asarray(w2[e], np.float32).astype(BF16) for e in range(E)]

    in_maps = []
    for c in range(NCORES):
        m = {}
        for j, B in enumerate(parts):
            M = B * P
            ent = asg.get((c, j))
            e = ent[0] if ent is not None else 0
            blk = np.zeros((M, D), np.float32)
            if ent is not None:
                _, b0, nb = ent
                blk[:nb * P] = xe[e][b0 * P:(b0 + nb) * P]
            # [DT, P, M]: xt[d, p, m] = blk[m, d*P+p]
            m[f"xt{j}"] = np.ascontiguousarray(
                blk.reshape(M, DT, P).transpose(1, 2, 0)).astype(BF16)
            m[f"w1p{j}"] = w1b[e]
            m[f"w3p{j}"] = w3b[e]
            m[f"w2p{j}"] = w2b[e]
        in_maps.append(m)

    kw = {}
    if os.environ.get("KERNEL_TRACE"):
        kw = dict(trace=True, tmpdir=os.environ.get("KERNEL_TRACE_DIR") or None)
    res = run_bass_kernel_spmd(nc, in_maps, core_ids=list(range(NCORES)), **kw)
    global _LAST_RESULT
    _LAST_RESULT = res

    # reassemble expert-grouped output stream, then scatter to token order
    outp = np.zeros((ltot, D), np.float32)
    for (c, j), (e, b0, nb) in asg.items():
        rows = res.results[c][f"out{j}"][:nb * P]
        s = m_off[e] + b0 * P
        stop = min(int(m_off[e] + m_sizes[e]), int(s + nb * P))
        if stop > s:
            outp[s:stop] = rows[:stop - s]

    out = np.zeros((T + 1, D), np.float32)
    out[perm[:ltot]] = outp
    return out[:T]
